# revision 50
# baseline (speedup 1.0000x reference)
"""Trainium2 Bass kernel for nn_CIE_89893665505337 (gnn_message_passing).

2x [MHA(global over 4096 nodes) + FF] transformer blocks + HypergraphConv.
8-core SPMD: nodes sharded 512/core, transposed activations hT [D=128, n],
fp16 matmul operands, f32 residual stream.

v2: hypergraph conv folded into a host-precomputed dense A' = Dinv H Binv H^T
matrix (sharded by columns) -> one local matmul chain + one ReduceScatter,
replacing two AllGathers + two incidence-matmul stages. Inter-layer
AllGather carried in fp8. Softmax exp split across Scalar/Vector/GpSimd
engines (Schraudolph fast-exp on the latter two).
"""
import os
import sys

for _p in ("/opt/trn_rl_repo", "/root/.axon_site/_ro/trn_rl_repo"):
    if os.path.isdir(_p) and _p not in sys.path:
        sys.path.insert(0, _p)

import numpy as np

import concourse.bacc as bacc
import concourse.bass as bass
import concourse.tile as tile
from concourse import mybir
from concourse.bass_utils import run_bass_kernel_spmd

F32 = mybir.dt.float32
F32R = mybir.dt.float32r
F16 = mybir.dt.float16
F8 = mybir.dt.float8e4
I16 = mybir.dt.int16
I32 = mybir.dt.int32
AF = mybir.ActivationFunctionType
ALU = mybir.AluOpType

W = 8            # cores
N = 4096         # nodes
D = 128          # model dim
H = 2            # heads
DH = 64          # head dim
FF = 256         # ff dim
NE = 2048        # hyperedges
NL = N // W      # 512 local nodes
EPS = 1e-5
NCH = N // 128   # 32 m-chunks
RSQRT_MAGIC = 0x5F3759DF

H_SZ = 128 * NL            # elems of hT_loc [128, 512]

# Schraudolph fast-exp constants for exp(s*0.125) in fp16 bit domain:
# i16 = round(s * 0.125*log2(e)*1024 + (15360 - 44)); bitcast -> fp16
FE_A = 0.125 * 1.4426950408889634 * 1024.0
FE_B = 15360.0 - 44.0

# packed fp16 weight-bundle column offsets
_WQ = 0
_WKV = _WQ + 128          # [Wk | Wv_ext(130, ones-col via bias)]
_WO = _WKV + 258
_W1 = _WO + 128
_W2 = _W1 + FF            # 0.5*W2 packed [128, 2*128]
_WH = _W2 + 256
_ID = _WH + 128
_ONE = _ID + 128          # ones col (1/128) for LN stats
_WF16_COLS = _ONE + 1

# packed f32 bundle: bq | Bkv(131) | bo | bf2 | bh | bf1h(2)
_BQ = 0
_BKV = 1
_BO = 132
_BF2 = 133
_BH = 134
_BF1 = 135
_WB32_COLS = 137

# packed [1, X] fp16 row bundle: ln1r(768) | ln2r(768) | w2sum(128) |
# ln1rn(128) | ln2rn(128)
_LN1R = 0
_LN2R = 768
_W2S = 1536
_LN1N = 1664
_LN2N = 1792
_LNR_COLS = 1920


def build_program():
    nc = bacc.Bacc("TRN2", target_bir_lowering=False, debug=False, num_devices=W)

    def inp(name, shape, dt=F32):
        return nc.dram_tensor(name, shape, dt, kind="ExternalInput")

    xT32 = inp("xT32", [128, NL])
    xT16 = inp("xT16", [128, NL], F16)
    xTf8 = inp("xTf8", [128, W, NL], F8)     # full x^T fp8, rank-blocked
    WQKV = inp("WQKV", [128, 386], F16)      # [Wq | Wk | Wv_ext] critical
    WF16 = inp("WF16", [128, _WF16_COLS - 386], F16)
    WB32 = inp("WB32", [128, _WB32_COLS])
    LNR = inp("LNR", [1, _LNR_COLS], F16)
    AT = inp("AT", [128, 4, N], F16)         # A'^T slice: [m_loc, n] chunked

    out_t = nc.dram_tensor("outT", [128, NL], F32, kind="ExternalOutput")

    # layer-2 h AllGather (fp8)
    kv_in = nc.dram_tensor("kv_in", [H_SZ], F8)
    kv_out = nc.dram_tensor("kv_out", [W, H_SZ], F8, addr_space="Shared")
    # conv partial ReduceScatter (fp16)
    rs_in = nc.dram_tensor("rs_in", [W, H_SZ], F16)
    rs_out = nc.dram_tensor("rs_out", [H_SZ], F16)

    RG = [list(range(W))]

    with tile.TileContext(nc) as tc:
        with (
            tc.tile_pool(name="wpool", bufs=1) as wp,      # persistent weights
            tc.tile_pool(name="sb", bufs=3) as sb,         # general sbuf tiles
            tc.tile_pool(name="kv", bufs=2) as kvp,        # kT/V per layer
            tc.tile_pool(name="expp", bufs=5) as expp,     # exp tiles
            tc.tile_pool(name="ps_s", bufs=4, space="PSUM") as ps_s,   # scores
            tc.tile_pool(name="ps_o", bufs=2, space="PSUM") as ps_o,   # attn acc
            tc.tile_pool(name="ps_m", bufs=2, space="PSUM") as ps_m,   # misc
        ):
            # ---- constant loads (few big bundles) ----
            hT16 = sb.tile([128, NL], F16, tag="hT16")
            nc.gpsimd.dma_start(hT16[:], xT16[:])

            # biases first (gate kT/qT/v epilogues), then critical weights,
            # then the full-x load, then the bulk weights
            wb = wp.tile([128, _WB32_COLS], F32, name="wb")
            nc.sync.dma_start(wb[:], WB32[:])
            wqkv = wp.tile([128, 386], F16, name="wqkv")
            nc.sync.dma_start(wqkv[:], WQKV[:])
            w_q = wqkv[:, 0:128]
            w_k = wqkv[:, 128:256]
            w_v = wqkv[:, 256:386]

            # layer-1 full-x load: attention-1 gate, highest priority
            hTf0 = kvp.tile([128, W, NL], F8, tag="hT_full", name="hTf0")
            nc.sync.dma_start(hTf0[:], xTf8[:])

            wf = wp.tile([128, _WF16_COLS - 386], F16, name="wf")
            nc.sync.dma_start(wf[:], WF16[:])
            w_o = wf[:, _WO - 386:_WO - 386 + 128]
            w_1 = wf[:, _W1 - 386:_W1 - 386 + FF]
            w_2 = [wf[:, _W2 - 386:_W2 - 386 + 128],
                   wf[:, _W2 - 258:_W2 - 258 + 128]]
            w_h = wf[:, _WH - 386:_WH - 386 + 128]
            c_id16 = wf[:, _ID - 386:_ID - 386 + 128]
            c_one16 = wf[:, _ONE - 386:_ONE - 386 + 1]
            c_bq = wb[:, _BQ:_BQ + 1]
            c_bk = wb[:, _BKV:_BKV + 1]
            c_bv = wb[:, _BKV + 1:_BKV + 131]
            c_bo = wb[:, _BO:_BO + 1]
            c_bf2 = wb[:, _BF2:_BF2 + 1]
            c_bh = wb[:, _BH:_BH + 1]
            c_bf1 = wb[:, _BF1:_BF1 + 2]

            lnr = wp.tile([1, _LNR_COLS], F16, name="lnr")
            nc.sync.dma_start(lnr[:], LNR[:])
            c_ln = [lnr[:, _LN1R:_LN1R + 768], lnr[:, _LN2R:_LN2R + 768]]
            c_w2s = lnr[:, _W2S:_W2S + 128]
            c_lnn = [lnr[:, _LN1N:_LN1N + 128], lnr[:, _LN2N:_LN2N + 128]]

            hT32 = sb.tile([128, NL], F32, tag="hT32")
            nc.gpsimd.dma_start(hT32[:], xT32[:])

            c_magic = wp.tile([128, 4], I32)
            nc.vector.memset(c_magic[:], RSQRT_MAGIC)
            c_ones1 = wp.tile([1, 64], F16)   # lhsT for den broadcast
            nc.vector.memset(c_ones1[:], 1.0)

            # conv A'^T slice (4MB): needed only at ~2/3 of the timeline
            at16 = wp.tile([128, 4, N], F16, name="at16")
            for c in range(4):
                nc.sync.dma_start(at16[:, c, :], AT[:, c, :])

            # ---------- helpers ----------
            CK = NL // 2          # tail column-chunk width (pipelined x2)

            def pool_for(ck):
                return (ps_m, "m") if ck == 0 else (ps_s, "scr")

            class LNState:
                pass

            def ln_stage1(t32c, ck, uid):
                """stats + rsqrt chain for one column chunk. t32c: [128, CK]."""
                st = LNState()
                st.t32c = t32c
                pool, ptag = pool_for(ck)
                veng = nc.vector
                st.t2q = sb.tile([128, CK], F16, tag=f"ln_t2_{ck}")
                with nc.allow_low_precision(reason="LN sq to fp16"):
                    nc.scalar.square(st.t2q[:], t32c[:])
                t16 = sb.tile([128, CK], F16, tag=f"ln_t16_{ck}")
                veng.tensor_copy(t16[:], t32c[:])
                st.stats = pool.tile([128, 4], F32, tag=ptag,
                                     name=f"st{uid}{ck}")
                for s in range(2):
                    nc.tensor.matmul(st.stats[:, s:s + 1],
                                     t16[:, s * 128:(s + 1) * 128], c_one16)
                    nc.tensor.matmul(st.stats[:, 2 + s:3 + s],
                                     st.t2q[:, s * 128:(s + 1) * 128], c_one16)
                return st

            def ln_stage2(st, ck, uid):
                veng = nc.vector
                m = st.stats[:, 0:2]
                msq = sb.tile([128, 2], F32, tag=f"ln_msq_{ck}")
                nc.scalar.square(msq[:], m)
                ve = sb.tile([128, 2], F32, tag=f"ln_ve_{ck}")
                nc.vector.scalar_tensor_tensor(ve[:], st.stats[:, 2:4], EPS,
                                               msq[:], ALU.add, ALU.subtract)
                sh = sb.tile([128, 2], I32, tag=f"ln_sh_{ck}")
                veng.tensor_scalar(sh[:], ve[:].bitcast(I32), 1, None,
                                   ALU.logical_shift_right)
                P = sb.tile([128, 4], F32, tag=f"ln_P_{ck}")
                y = P[:, 0:2]
                veng.tensor_tensor(y.bitcast(I32), c_magic[:, 0:2], sh[:],
                                   ALU.subtract)
                a = sb.tile([128, 2], F32, tag=f"ln_a_{ck}")
                veng.tensor_tensor(a[:], y, y, ALU.mult)
                veng.scalar_tensor_tensor(a[:], a[:], -0.5, ve[:],
                                          ALU.mult, ALU.mult)
                veng.scalar_tensor_tensor(y, a[:], 1.5, y,
                                          ALU.add, ALU.mult)
                nc.vector.tensor_tensor(P[:, 2:4], m, y, ALU.mult)  # m*inv
                P16 = sb.tile([128, 4], F16, tag=f"ln_P16_{ck}")
                veng.tensor_copy(P16[:], P[:])
                st.P16 = P16

            def ln_stage3(st, ck, uid, lnc, lncn):
                pool, ptag = pool_for(ck)
                psTa = pool.tile([1, CK], F32, tag=ptag, name=f"ta{uid}{ck}")
                psTb = pool.tile([1, CK], F32, tag=ptag, name=f"tb{uid}{ck}")
                for s in range(2):
                    nc.tensor.matmul(psTa[:, s * 128:(s + 1) * 128],
                                     st.P16[:, s:s + 1], c_id16)
                    nc.tensor.matmul(psTb[:, s * 128:(s + 1) * 128],
                                     st.P16[:, 2 + s:3 + s], c_id16)
                T = sb.tile([1, 2 * CK], F16, tag=f"ln_T_{ck}")
                nc.scalar.copy(T[0:1, 0:CK], psTa[:])
                with nc.allow_low_precision(reason="LN strip fp16"):
                    nc.vector.tensor_copy(T[0:1, CK:2 * CK], psTb[:])
                st.psA = pool.tile([128, CK], F32, tag=ptag, name=f"pa{uid}{ck}")
                st.psB = pool.tile([128, CK], F32, tag=ptag, name=f"pb{uid}{ck}")
                nc.tensor.matmul(st.psA[:], lnc[0:1, 0:128], T[0:1, 0:CK])
                nc.tensor.matmul(st.psB[:], lncn[0:1, :], T[0:1, CK:2 * CK],
                                 start=True, stop=False)
                nc.tensor.matmul(st.psB[:], lnc[0:1, 128:256],
                                 lnc[0:1, 256:256 + CK], start=False, stop=True)

            def ln_stage4(st, ck, out32, out16, out8, cols):
                u = sb.tile([128, CK], F32, tag=f"ln_u_{ck}")
                nc.vector.tensor_tensor(u[:], st.t32c[:], st.psA[:], ALU.mult)
                nc.vector.tensor_tensor(out32[:, cols], u[:], st.psB[:], ALU.add)
                with nc.allow_low_precision(reason="h16 copy"):
                    nc.scalar.copy(out16[:, cols], out32[:, cols])
                if out8 is not None:
                    with nc.allow_low_precision(reason="h8 copy"):
                        nc.vector.tensor_copy(out8[:, cols], out32[:, cols])

            # ---------- transformer layer ----------
            def mha_ff_layer(li, hT32_in, hT16_in, hTf_pre, h8_in,
                             tail_hook=None):
                if hTf_pre is not None:
                    hTf = hTf_pre
                else:
                    hTf = kvp.tile([128, W, NL], F8, tag="hT_full")
                    nc.sync.dma_start(
                        kv_in[:].rearrange("(p j) -> p j", p=128), h8_in[:])
                    nc.gpsimd.collective_compute(
                        "AllGather", ALU.bypass, replica_groups=RG,
                        ins=[kv_in[:]], outs=[kv_out[:]])

                ps_q = ps_m.tile([128, NL], F32, tag="m")
                nc.tensor.matmul(ps_q[:], w_q, hT16_in[:])
                qT = sb.tile([128, NL], F16, tag="qT")
                with nc.allow_low_precision(reason="qT fp16"):
                    nc.scalar.activation(qT[:], ps_q[:], AF.Identity, bias=c_bq)

                if hTf_pre is None:
                    nc.sync.dma_start(
                        hTf[:, 0:4, :],
                        kv_out[0:4, :].rearrange("w (p j) -> p w j", p=128))
                    nc.sync.dma_start(
                        hTf[:, 4:8, :],
                        kv_out[4:8, :].rearrange("w (p j) -> p w j", p=128))
                kT_sb = kvp.tile([128, W, NL], F16, tag="kT_full")
                v_sb = kvp.tile([128, NCH, 130], F16, tag="v_full")
                for r in range(W):
                    ps_k = ps_m.tile([128, NL], F32, tag="m")
                    nc.tensor.matmul(ps_k[:], w_k, hTf[:, r, :])
                    with nc.allow_low_precision(reason="kT fp16"):
                        nc.scalar.activation(kT_sb[:, r, :], ps_k[:],
                                             AF.Identity, bias=c_bk)
                    for cc in range(4):
                        c = 4 * r + cc
                        pv = ps_m.tile([128, 130], F32, tag="m")
                        nc.tensor.matmul(
                            pv[:], hTf[:, r, cc * 128:(cc + 1) * 128], w_v)
                        with nc.allow_low_precision(reason="v fp16"):
                            nc.vector.tensor_tensor(v_sb[:, c, :], pv[:], c_bv,
                                                    ALU.add)

                # attention: per-chunk groups with 1-group QK lookahead so
                # QK(c+1) overlaps exp(c); exp split across Act / DVE
                oT = sb.tile([128, NL], F16, tag="oT")
                po = [ps_o.tile([65, NL], F32, tag="o_acc", name=f"po{li}_{h}")
                      for h in range(H)]

                def emit_qk(c):
                    r, cc = c // 4, c % 4
                    ts = []
                    for h in range(H):
                        hs = slice(h * 64, (h + 1) * 64)
                        t = ps_s.tile([128, NL], F32, tag="scr",
                                      name=f"scr{li}_{c}_{h}")
                        nc.tensor.matmul(t[:],
                                         kT_sb[hs, r, cc * 128:(cc + 1) * 128],
                                         qT[hs, :])
                        ts.append(t)
                    return ts

                pscr_next = emit_qk(0)
                for c in range(NCH):
                    pscr = pscr_next
                    exs = []
                    for h in range(H):
                        if h == 0 or (c % 4 == 3 and c < 24):
                            ex = expp.tile([128, NL], F16, tag="exp",
                                           name=f"ex{li}_{c}_{h}")
                            nc.scalar.activation(ex[:], pscr[h][:], AF.Exp,
                                                 scale=0.125)
                            exs.append(ex[:])
                        else:
                            exi = expp.tile([128, NL], I16, tag="exp",
                                            name=f"ex{li}_{c}_{h}")
                            nc.vector.tensor_scalar(exi[:], pscr[h][:], FE_A,
                                                    FE_B, ALU.mult, ALU.add)
                            exs.append(exi[:].bitcast(F16))
                    if c + 1 < NCH:
                        pscr_next = emit_qk(c + 1)
                    for h in range(H):
                        nc.tensor.matmul(
                            po[h][:], v_sb[:, c, 65 * h:65 * h + 65],
                            exs[h][:], start=(c == 0), stop=(c == NCH - 1))
                # attention normalize: per-head recip + broadcast (unchunked)
                pdens = []
                for h in range(H):
                    den32 = sb.tile([1, NL], F32, tag="den32",
                                    name=f"den{li}_{h}")
                    nc.scalar.copy(den32[:], po[h][64:65, :])
                    rden32 = sb.tile([1, NL], F32, tag="rden32",
                                     name=f"rden{li}_{h}")
                    nc.vector.reciprocal_approx_fast(rden32[:], den32[:])
                    den16 = sb.tile([1, NL], F16, tag="den16",
                                    name=f"rden16_{li}_{h}")
                    with nc.allow_low_precision(reason="attn denom fp16"):
                        nc.scalar.copy(den16[:], rden32[:])
                    pden = ps_s.tile([64, NL], F32, tag="scr",
                                     name=f"pden{li}_{h}")
                    nc.tensor.matmul(pden[:], c_ones1[:], den16[:])
                    denB = sb.tile([64, NL], F16, tag="denB",
                                   name=f"denB{li}_{h}")
                    with nc.allow_low_precision(reason="denB fp16"):
                        nc.vector.tensor_copy(denB[:], pden[:])
                    pdens.append(denB)

                # ---- pipelined tail: 2 column chunks, stage-interleaved ----
                h1_16 = sb.tile([128, NL], F16, tag="h1_16")
                h1_32 = sb.tile([128, NL], F32, tag="h1_32")
                h2_16 = sb.tile([128, NL], F16, tag="hT16")
                h2_32 = sb.tile([128, NL], F32, tag="h2_32")
                h2_8 = None
                if li == 0:
                    h2_8 = sb.tile([128, NL], F8, tag="hT8")

                COLS = [slice(0, CK), slice(CK, 2 * CK)]
                t1c, ln1, zt, t2c, ln2 = {}, {}, {}, {}, {}

                for ck in range(2):
                    cols = COLS[ck]
                    with nc.allow_low_precision(reason="attn normalize fp16"):
                        for h in range(H):
                            hs = slice(h * 64, (h + 1) * 64)
                            nc.vector.tensor_tensor(oT[hs, cols],
                                                    po[h][0:64, cols],
                                                    pdens[h][:, cols],
                                                    ALU.mult)
                for ck in range(2):
                    cols = COLS[ck]
                    pool, ptag = pool_for(ck)
                    ps_p = pool.tile([128, CK], F32, tag=ptag,
                                     name=f"pp{li}{ck}")
                    nc.tensor.matmul(ps_p[:], w_o, oT[:, cols])
                    t1 = sb.tile([128, CK], F32, tag=f"resid1_{ck}")
                    nc.vector.scalar_tensor_tensor(t1[:], ps_p[:], c_bo,
                                                   hT32_in[:, cols],
                                                   ALU.add, ALU.add)
                    t1c[ck] = t1
                for ck in range(2):
                    ln1[ck] = ln_stage1(t1c[ck], ck, f"a{li}")
                for ck in range(2):
                    ln_stage2(ln1[ck], ck, f"a{li}")
                for ck in range(2):
                    ln_stage3(ln1[ck], ck, f"a{li}", c_ln[0], c_lnn[0])
                for ck in range(2):
                    ln_stage4(ln1[ck], ck, h1_32, h1_16, None, COLS[ck])
                # FF: sigmoid = 0.5*tanh(0.5x+0.5*bf1)+0.5 folded into 0.5*W2
                for ck in range(2):
                    cols = COLS[ck]
                    pool, ptag = pool_for(ck)
                    z = sb.tile([128, 2, CK], F16, tag=f"z_{ck}")
                    for f in range(2):
                        pz = pool.tile([128, CK], F32, tag=ptag,
                                       name=f"pz{li}{ck}{f}")
                        nc.tensor.matmul(pz[:], w_1[:, f * 128:(f + 1) * 128],
                                         h1_16[:, cols])
                        nc.scalar.activation(z[:, f, :], pz[:], AF.Tanh,
                                             bias=c_bf1[:, f:f + 1], scale=0.5)
                    zt[ck] = z
                for ck in range(2):
                    cols = COLS[ck]
                    pool, ptag = pool_for(ck)
                    ps_f = pool.tile([128, CK], F32, tag=ptag,
                                     name=f"pf{li}{ck}")
                    nc.tensor.matmul(ps_f[:], w_2[0], zt[ck][:, 0, :],
                                     start=True, stop=False)
                    nc.tensor.matmul(ps_f[:], w_2[1], zt[ck][:, 1, :],
                                     start=False, stop=False)
                    nc.tensor.matmul(ps_f[:], c_w2s, c_ln[0][0:1, 256:256 + CK],
                                     start=False, stop=True)
                    t2 = sb.tile([128, CK], F32, tag=f"resid2_{ck}")
                    nc.vector.scalar_tensor_tensor(t2[:], ps_f[:], c_bf2,
                                                   h1_32[:, cols],
                                                   ALU.add, ALU.add)
                    t2c[ck] = t2
                for ck in range(2):
                    ln2[ck] = ln_stage1(t2c[ck], ck, f"b{li}")
                for ck in range(2):
                    ln_stage2(ln2[ck], ck, f"b{li}")
                for ck in range(2):
                    ln_stage3(ln2[ck], ck, f"b{li}", c_ln[1], c_lnn[1])
                for ck in range(2):
                    ln_stage4(ln2[ck], ck, h2_32, h2_16, h2_8, COLS[ck])
                    if tail_hook is not None:
                        tail_hook(ck, h2_16)
                return h2_32, h2_16, h2_8

            # conv: outT = relu(A' @ (h @ Wh) + bh), partial A-matmuls woven
            # into layer-2's tail as h2_16 column chunks complete
            conv = {}

            def conv_hook(ck, h16t):
                if ck == 0:
                    xt_sb = sb.tile([128, 4, 128], F16, tag="xt_loc")
                    conv["xt"] = xt_sb
                    for c in (0, 1):
                        px = ps_m.tile([128, 128], F32, tag="m", name=f"px{c}")
                        nc.tensor.matmul(px[:], h16t[:, c * 128:(c + 1) * 128],
                                         w_h)
                        nc.vector.tensor_copy(xt_sb[:, c, :], px[:])
                    ppA = [ps_s.tile([128, NL], F32, tag="scr", name=f"ppA{b}")
                           for b in range(4)]
                    conv["ppA"] = ppA
                    for b in range(4):
                        for c in (0, 1):
                            nc.tensor.matmul(ppA[b][:], xt_sb[:, c, :],
                                             at16[:, c, b * NL:(b + 1) * NL],
                                             start=(c == 0), stop=False)
                    return
                xt_sb = conv["xt"]
                for c in (2, 3):
                    px = ps_m.tile([128, 128], F32, tag="m", name=f"px{c}")
                    nc.tensor.matmul(px[:], h16t[:, c * 128:(c + 1) * 128], w_h)
                    nc.vector.tensor_copy(xt_sb[:, c, :], px[:])
                partial = sb.tile([128, W, NL], F16, tag="partial")
                ppA = conv["ppA"]
                for b in range(4):
                    for c in (2, 3):
                        nc.tensor.matmul(ppA[b][:], xt_sb[:, c, :],
                                         at16[:, c, b * NL:(b + 1) * NL],
                                         start=False, stop=(c == 3))
                ppB = [ps_s.tile([128, NL], F32, tag="scr", name=f"ppB{b}")
                       for b in range(4)]
                for b in range(4):
                    with nc.allow_low_precision(reason="conv partial fp16"):
                        if b % 2 == 0:
                            nc.scalar.copy(partial[:, b, :], ppA[b][:])
                        else:
                            nc.vector.tensor_copy(partial[:, b, :], ppA[b][:])
                    nc.sync.dma_start(
                        rs_in[b, :].rearrange("(p j) -> p j", p=128),
                        partial[:, b, :])
                for b in range(4):
                    for c in range(4):
                        nc.tensor.matmul(ppB[b][:], xt_sb[:, c, :],
                                         at16[:, c, (4 + b) * NL:(5 + b) * NL],
                                         start=(c == 0), stop=(c == 3))
                for b in range(4):
                    with nc.allow_low_precision(reason="conv partial fp16"):
                        if b % 2 == 0:
                            nc.scalar.copy(partial[:, 4 + b, :], ppB[b][:])
                        else:
                            nc.vector.tensor_copy(partial[:, 4 + b, :],
                                                  ppB[b][:])
                    nc.sync.dma_start(
                        rs_in[4 + b, :].rearrange("(p j) -> p j", p=128),
                        partial[:, 4 + b, :])

            h32, h16, h8 = mha_ff_layer(0, hT32, hT16, hTf0, None)
            h32, h16, _ = mha_ff_layer(1, h32, h16, None, h8,
                                       tail_hook=conv_hook)

            nc.gpsimd.collective_compute(
                "ReduceScatter", ALU.add, replica_groups=RG,
                ins=[rs_in[:, :]], outs=[rs_out[:]])

            convo = sb.tile([128, NL], F16, tag="convo")
            nc.sync.dma_start(
                convo[:], rs_out[:].rearrange("(p j) -> p j", p=128))
            res = sb.tile([128, NL], F32, tag="res")
            nc.scalar.activation(res[:], convo[:], AF.Relu, bias=c_bh)
            nc.sync.dma_start(out_t[:], res[:])

    nc.compile()
    return nc


_NC = None


def _get_nc():
    global _NC
    if _NC is None:
        _NC = build_program()
    return _NC


def make_in_maps(inputs):
    import ml_dtypes
    F8NP = ml_dtypes.float8_e4m3

    x = np.asarray(inputs["x"], dtype=np.float32)
    edge = np.asarray(inputs["edge"])
    gw = {k: np.asarray(inputs[k], dtype=np.float32) for k in
          ("Wq", "bq", "Wk", "bk", "Wv", "bv", "Wo", "bo", "g_ln1", "b_ln1",
           "W1", "bf1", "W2", "bf2", "g_ln2", "b_ln2", "Wh", "bh")}

    node_idx = np.asarray(edge[0], dtype=np.int64)
    he_idx = np.asarray(edge[1], dtype=np.int64)
    counts = np.zeros((N, NE), dtype=np.float32)
    np.add.at(counts, (node_idx, he_idx), 1.0)
    Bdeg = counts.sum(axis=0)
    Ddeg = counts.sum(axis=1)
    Binv = np.where(Bdeg > 0, 1.0 / np.maximum(Bdeg, 1), 0.0).astype(np.float32)
    Dinv = np.where(Ddeg > 0, 1.0 / np.maximum(Ddeg, 1), 0.0).astype(np.float32)

    # S = H Binv H^T (symmetric); per-core slice of A'^T = S[rows,:] * Dinv
    S = (counts * Binv[None, :]) @ counts.T

    wqkv = np.zeros((128, 386), dtype=np.float16)
    wqkv[:, 0:128] = gw["Wq"]
    wqkv[:, 128:256] = gw["Wk"]
    wqkv[:, 256:320] = gw["Wv"][:, 0:64]
    wqkv[:, 321:385] = gw["Wv"][:, 64:128]
    wf16 = np.zeros((128, _WF16_COLS), dtype=np.float16)
    wf16[:, _WO:_WO + 128] = gw["Wo"]
    wf16[:, _W1:_W1 + FF] = gw["W1"]
    wf16[:, _W2:_W2 + 256] = (
        (0.5 * gw["W2"]).reshape(2, 128, 128).transpose(1, 0, 2).reshape(128, 256))
    wf16[:, _WH:_WH + 128] = gw["Wh"]
    wf16[:, _ID:_ID + 128] = np.eye(128, dtype=np.float16)
    wf16[:, _ONE:_ONE + 1] = 1.0 / 128

    wb32 = np.zeros((128, _WB32_COLS), dtype=np.float32)
    wb32[:, _BQ] = gw["bq"]
    wb32[:, _BKV] = gw["bk"]
    wb32[:, _BKV + 1:_BKV + 65] = gw["bv"][0:64][None, :]
    wb32[:, _BKV + 66:_BKV + 130] = gw["bv"][64:128][None, :]
    wb32[:, _BKV + 65] = 1.0
    wb32[:, _BKV + 130] = 1.0
    wb32[:, _BO] = gw["bo"]
    wb32[:, _BF2] = gw["bf2"]
    wb32[:, _BH] = gw["bh"]
    wb32[:, _BF1:_BF1 + 2] = (0.5 * gw["bf1"]).reshape(2, 128).T

    def lnrows(g, b):
        r = np.zeros(768, dtype=np.float16)
        r[0:128] = g
        r[128:256] = b
        r[256:768] = 1.0
        return r

    lnr = np.zeros((1, _LNR_COLS), dtype=np.float16)
    lnr[0, _LN1R:_LN1R + 768] = lnrows(gw["g_ln1"], gw["b_ln1"])
    lnr[0, _LN2R:_LN2R + 768] = lnrows(gw["g_ln2"], gw["b_ln2"])
    lnr[0, _W2S:_W2S + 128] = 0.5 * gw["W2"].sum(axis=0)
    lnr[0, _LN1N:_LN1N + 128] = -gw["g_ln1"]
    lnr[0, _LN2N:_LN2N + 128] = -gw["g_ln2"]

    xTf8 = np.ascontiguousarray(
        x.T.astype(np.float16).reshape(128, W, NL)).astype(F8NP)

    shared = dict(WQKV=wqkv, WF16=np.ascontiguousarray(wf16[:, 386:]),
                  WB32=wb32, LNR=lnr, xTf8=xTf8)

    in_maps = []
    for r in range(W):
        rows = slice(r * NL, (r + 1) * NL)
        xT = np.ascontiguousarray(x[rows, :].T)
        at = np.ascontiguousarray(
            (S[rows, :] * Dinv[None, :]).astype(np.float16)
            .reshape(4, 128, N).transpose(1, 0, 2))
        m = dict(shared)
        m.update(xT32=xT.astype(np.float32), xT16=xT.astype(np.float16),
                 AT=at)
        in_maps.append(m)
    return in_maps


def kernel(**inputs) -> np.ndarray:
    nc = _get_nc()
    in_maps = make_in_maps(inputs)
    res = run_bass_kernel_spmd(nc, in_maps, core_ids=list(range(W)))
    out = np.empty((N, D), dtype=np.float32)
    for r in range(W):
        out[r * NL:(r + 1) * NL, :] = res.results[r]["outT"].T
    return out


if __name__ == "__main__":
    build_program()
    print("build OK")


# revision 51
# speedup vs baseline: 1.0008x; 1.0008x over previous
"""Trainium2 Bass kernel for nn_CIE_89893665505337 (gnn_message_passing).

2x [MHA(global over 4096 nodes) + FF] transformer blocks + HypergraphConv.
8-core SPMD: nodes sharded 512/core, transposed activations hT [D=128, n],
fp16 matmul operands, f32 residual stream.

v2: hypergraph conv folded into a host-precomputed dense A' = Dinv H Binv H^T
matrix (sharded by columns) -> one local matmul chain + one ReduceScatter,
replacing two AllGathers + two incidence-matmul stages. Inter-layer
AllGather carried in fp8. Softmax exp split across Scalar/Vector/GpSimd
engines (Schraudolph fast-exp on the latter two).
"""
import os
import sys

for _p in ("/opt/trn_rl_repo", "/root/.axon_site/_ro/trn_rl_repo"):
    if os.path.isdir(_p) and _p not in sys.path:
        sys.path.insert(0, _p)

import numpy as np

import concourse.bacc as bacc
import concourse.bass as bass
import concourse.tile as tile
from concourse import mybir
from concourse.bass_utils import run_bass_kernel_spmd

F32 = mybir.dt.float32
F32R = mybir.dt.float32r
F16 = mybir.dt.float16
F8 = mybir.dt.float8e4
I16 = mybir.dt.int16
I32 = mybir.dt.int32
AF = mybir.ActivationFunctionType
ALU = mybir.AluOpType

W = 8            # cores
N = 4096         # nodes
D = 128          # model dim
H = 2            # heads
DH = 64          # head dim
FF = 256         # ff dim
NE = 2048        # hyperedges
NL = N // W      # 512 local nodes
EPS = 1e-5
NCH = N // 128   # 32 m-chunks
RSQRT_MAGIC = 0x5F3759DF

H_SZ = 128 * NL            # elems of hT_loc [128, 512]

# Schraudolph fast-exp constants for exp(s*0.125) in fp16 bit domain:
# i16 = round(s * 0.125*log2(e)*1024 + (15360 - 44)); bitcast -> fp16
FE_A = 0.125 * 1.4426950408889634 * 1024.0
FE_B = 15360.0 - 44.0

# packed fp16 weight-bundle column offsets
_WQ = 0
_WKV = _WQ + 128          # [Wk | Wv_ext(130, ones-col via bias)]
_WO = _WKV + 258
_W1 = _WO + 128
_W2 = _W1 + FF            # 0.5*W2 packed [128, 2*128]
_WH = _W2 + 256
_ID = _WH + 128
_ONE = _ID + 128          # ones col (1/128) for LN stats
_WF16_COLS = _ONE + 1

# packed f32 bundle: bq | Bkv(131) | bo | bf2 | bh | bf1h(2)
_BQ = 0
_BKV = 1
_BO = 132
_BF2 = 133
_BH = 134
_BF1 = 135
_WB32_COLS = 137

# packed [1, X] fp16 row bundle: ln1r(768) | ln2r(768) | w2sum(128) |
# ln1rn(128) | ln2rn(128)
_LN1R = 0
_LN2R = 768
_W2S = 1536
_LN1N = 1664
_LN2N = 1792
_LNR_COLS = 1920


def build_program():
    nc = bacc.Bacc("TRN2", target_bir_lowering=False, debug=False, num_devices=W)

    def inp(name, shape, dt=F32):
        return nc.dram_tensor(name, shape, dt, kind="ExternalInput")

    xT32 = inp("xT32", [128, NL])
    xT16 = inp("xT16", [128, NL], F16)
    xTf8 = inp("xTf8", [128, W, NL], F8)     # full x^T fp8, rank-blocked
    WQKV = inp("WQKV", [128, 386], F16)      # [Wq | Wk | Wv_ext] critical
    WF16 = inp("WF16", [128, _WF16_COLS - 386], F16)
    WB32 = inp("WB32", [128, _WB32_COLS])
    LNR = inp("LNR", [1, _LNR_COLS], F16)
    AT = inp("AT", [128, 4, N], F16)         # A'^T slice: [m_loc, n] chunked

    out_t = nc.dram_tensor("outT", [2, 128, NL // 2], F32,
                           kind="ExternalOutput")

    # layer-2 h AllGather (fp8)
    kv_in = nc.dram_tensor("kv_in", [H_SZ], F8)
    kv_out = nc.dram_tensor("kv_out", [W, H_SZ], F8, addr_space="Shared")
    # conv partial ReduceScatter (fp16)
    rs_in = nc.dram_tensor("rs_in", [W, H_SZ], F16)
    rs_out = nc.dram_tensor("rs_out", [2, H_SZ // 2], F16)

    RG = [list(range(W))]

    with tile.TileContext(nc) as tc:
        with (
            tc.tile_pool(name="wpool", bufs=1) as wp,      # persistent weights
            tc.tile_pool(name="sb", bufs=3) as sb,         # general sbuf tiles
            tc.tile_pool(name="kv", bufs=2) as kvp,        # kT/V per layer
            tc.tile_pool(name="expp", bufs=5) as expp,     # exp tiles
            tc.tile_pool(name="ps_s", bufs=4, space="PSUM") as ps_s,   # scores
            tc.tile_pool(name="ps_o", bufs=2, space="PSUM") as ps_o,   # attn acc
            tc.tile_pool(name="ps_m", bufs=2, space="PSUM") as ps_m,   # misc
        ):
            # ---- constant loads (few big bundles) ----
            hT16 = sb.tile([128, NL], F16, tag="hT16")
            nc.gpsimd.dma_start(hT16[:], xT16[:])

            # biases first (gate kT/qT/v epilogues), then critical weights,
            # then the full-x load, then the bulk weights
            wb = wp.tile([128, _WB32_COLS], F32, name="wb")
            nc.sync.dma_start(wb[:], WB32[:])
            wqkv = wp.tile([128, 386], F16, name="wqkv")
            nc.sync.dma_start(wqkv[:], WQKV[:])
            w_q = wqkv[:, 0:128]
            w_k = wqkv[:, 128:256]
            w_v = wqkv[:, 256:386]

            # layer-1 full-x load: attention-1 gate, highest priority
            hTf0 = kvp.tile([128, W, NL], F8, tag="hT_full", name="hTf0")
            nc.sync.dma_start(hTf0[:], xTf8[:])

            wf = wp.tile([128, _WF16_COLS - 386], F16, name="wf")
            nc.sync.dma_start(wf[:], WF16[:])
            w_o = wf[:, _WO - 386:_WO - 386 + 128]
            w_1 = wf[:, _W1 - 386:_W1 - 386 + FF]
            w_2 = [wf[:, _W2 - 386:_W2 - 386 + 128],
                   wf[:, _W2 - 258:_W2 - 258 + 128]]
            w_h = wf[:, _WH - 386:_WH - 386 + 128]
            c_id16 = wf[:, _ID - 386:_ID - 386 + 128]
            c_one16 = wf[:, _ONE - 386:_ONE - 386 + 1]
            c_bq = wb[:, _BQ:_BQ + 1]
            c_bk = wb[:, _BKV:_BKV + 1]
            c_bv = wb[:, _BKV + 1:_BKV + 131]
            c_bo = wb[:, _BO:_BO + 1]
            c_bf2 = wb[:, _BF2:_BF2 + 1]
            c_bh = wb[:, _BH:_BH + 1]
            c_bf1 = wb[:, _BF1:_BF1 + 2]

            lnr = wp.tile([1, _LNR_COLS], F16, name="lnr")
            nc.sync.dma_start(lnr[:], LNR[:])
            c_ln = [lnr[:, _LN1R:_LN1R + 768], lnr[:, _LN2R:_LN2R + 768]]
            c_w2s = lnr[:, _W2S:_W2S + 128]
            c_lnn = [lnr[:, _LN1N:_LN1N + 128], lnr[:, _LN2N:_LN2N + 128]]

            hT32 = sb.tile([128, NL], F32, tag="hT32")
            nc.gpsimd.dma_start(hT32[:], xT32[:])

            c_magic = wp.tile([128, 4], I32)
            nc.vector.memset(c_magic[:], RSQRT_MAGIC)
            c_ones1 = wp.tile([1, 64], F16)   # lhsT for den broadcast
            nc.vector.memset(c_ones1[:], 1.0)

            # conv A'^T slice (4MB): needed only at ~2/3 of the timeline
            at16 = wp.tile([128, 4, N], F16, name="at16")
            for c in range(4):
                nc.sync.dma_start(at16[:, c, :], AT[:, c, :])

            # ---------- helpers ----------
            CK = NL // 2          # tail column-chunk width (pipelined x2)

            def pool_for(ck):
                return (ps_m, "m") if ck == 0 else (ps_s, "scr")

            class LNState:
                pass

            def ln_stage1(t32c, ck, uid):
                """stats + rsqrt chain for one column chunk. t32c: [128, CK]."""
                st = LNState()
                st.t32c = t32c
                pool, ptag = pool_for(ck)
                veng = nc.vector
                st.t2q = sb.tile([128, CK], F16, tag=f"ln_t2_{ck}")
                with nc.allow_low_precision(reason="LN sq to fp16"):
                    nc.scalar.square(st.t2q[:], t32c[:])
                t16 = sb.tile([128, CK], F16, tag=f"ln_t16_{ck}")
                veng.tensor_copy(t16[:], t32c[:])
                st.stats = pool.tile([128, 4], F32, tag=ptag,
                                     name=f"st{uid}{ck}")
                for s in range(2):
                    nc.tensor.matmul(st.stats[:, s:s + 1],
                                     t16[:, s * 128:(s + 1) * 128], c_one16)
                    nc.tensor.matmul(st.stats[:, 2 + s:3 + s],
                                     st.t2q[:, s * 128:(s + 1) * 128], c_one16)
                return st

            def ln_stage2(st, ck, uid):
                veng = nc.vector
                m = st.stats[:, 0:2]
                msq = sb.tile([128, 2], F32, tag=f"ln_msq_{ck}")
                nc.scalar.square(msq[:], m)
                ve = sb.tile([128, 2], F32, tag=f"ln_ve_{ck}")
                nc.vector.scalar_tensor_tensor(ve[:], st.stats[:, 2:4], EPS,
                                               msq[:], ALU.add, ALU.subtract)
                sh = sb.tile([128, 2], I32, tag=f"ln_sh_{ck}")
                veng.tensor_scalar(sh[:], ve[:].bitcast(I32), 1, None,
                                   ALU.logical_shift_right)
                P = sb.tile([128, 4], F32, tag=f"ln_P_{ck}")
                y = P[:, 0:2]
                veng.tensor_tensor(y.bitcast(I32), c_magic[:, 0:2], sh[:],
                                   ALU.subtract)
                a = sb.tile([128, 2], F32, tag=f"ln_a_{ck}")
                veng.tensor_tensor(a[:], y, y, ALU.mult)
                veng.scalar_tensor_tensor(a[:], a[:], -0.5, ve[:],
                                          ALU.mult, ALU.mult)
                veng.scalar_tensor_tensor(y, a[:], 1.5, y,
                                          ALU.add, ALU.mult)
                nc.vector.tensor_tensor(P[:, 2:4], m, y, ALU.mult)  # m*inv
                P16 = sb.tile([128, 4], F16, tag=f"ln_P16_{ck}")
                veng.tensor_copy(P16[:], P[:])
                st.P16 = P16

            def ln_stage3(st, ck, uid, lnc, lncn):
                pool, ptag = pool_for(ck)
                psTa = pool.tile([1, CK], F32, tag=ptag, name=f"ta{uid}{ck}")
                psTb = pool.tile([1, CK], F32, tag=ptag, name=f"tb{uid}{ck}")
                for s in range(2):
                    nc.tensor.matmul(psTa[:, s * 128:(s + 1) * 128],
                                     st.P16[:, s:s + 1], c_id16)
                    nc.tensor.matmul(psTb[:, s * 128:(s + 1) * 128],
                                     st.P16[:, 2 + s:3 + s], c_id16)
                T = sb.tile([1, 2 * CK], F16, tag=f"ln_T_{ck}")
                nc.scalar.copy(T[0:1, 0:CK], psTa[:])
                with nc.allow_low_precision(reason="LN strip fp16"):
                    nc.vector.tensor_copy(T[0:1, CK:2 * CK], psTb[:])
                st.psA = pool.tile([128, CK], F32, tag=ptag, name=f"pa{uid}{ck}")
                st.psB = pool.tile([128, CK], F32, tag=ptag, name=f"pb{uid}{ck}")
                nc.tensor.matmul(st.psA[:], lnc[0:1, 0:128], T[0:1, 0:CK])
                nc.tensor.matmul(st.psB[:], lncn[0:1, :], T[0:1, CK:2 * CK],
                                 start=True, stop=False)
                nc.tensor.matmul(st.psB[:], lnc[0:1, 128:256],
                                 lnc[0:1, 256:256 + CK], start=False, stop=True)

            def ln_stage4(st, ck, out32, out16, out8, cols):
                u = sb.tile([128, CK], F32, tag=f"ln_u_{ck}")
                nc.vector.tensor_tensor(u[:], st.t32c[:], st.psA[:], ALU.mult)
                nc.vector.tensor_tensor(out32[:, cols], u[:], st.psB[:], ALU.add)
                with nc.allow_low_precision(reason="h16 copy"):
                    nc.scalar.copy(out16[:, cols], out32[:, cols])
                if out8 is not None:
                    with nc.allow_low_precision(reason="h8 copy"):
                        nc.vector.tensor_copy(out8[:, cols], out32[:, cols])

            # ---------- transformer layer ----------
            def mha_ff_layer(li, hT32_in, hT16_in, hTf_pre, h8_in,
                             tail_hook=None):
                if hTf_pre is not None:
                    hTf = hTf_pre
                else:
                    hTf = kvp.tile([128, W, NL], F8, tag="hT_full")
                    nc.sync.dma_start(
                        kv_in[:].rearrange("(p j) -> p j", p=128), h8_in[:])
                    nc.gpsimd.collective_compute(
                        "AllGather", ALU.bypass, replica_groups=RG,
                        ins=[kv_in[:]], outs=[kv_out[:]])

                ps_q = ps_m.tile([128, NL], F32, tag="m")
                nc.tensor.matmul(ps_q[:], w_q, hT16_in[:])
                qT = sb.tile([128, NL], F16, tag="qT")
                with nc.allow_low_precision(reason="qT fp16"):
                    nc.scalar.activation(qT[:], ps_q[:], AF.Identity, bias=c_bq)

                if hTf_pre is None:
                    nc.sync.dma_start(
                        hTf[:, 0:4, :],
                        kv_out[0:4, :].rearrange("w (p j) -> p w j", p=128))
                    nc.sync.dma_start(
                        hTf[:, 4:8, :],
                        kv_out[4:8, :].rearrange("w (p j) -> p w j", p=128))
                kT_sb = kvp.tile([128, W, NL], F16, tag="kT_full")
                v_sb = kvp.tile([128, NCH, 130], F16, tag="v_full")
                for r in range(W):
                    ps_k = ps_m.tile([128, NL], F32, tag="m")
                    nc.tensor.matmul(ps_k[:], w_k, hTf[:, r, :])
                    with nc.allow_low_precision(reason="kT fp16"):
                        nc.scalar.activation(kT_sb[:, r, :], ps_k[:],
                                             AF.Identity, bias=c_bk)
                    for cc in range(4):
                        c = 4 * r + cc
                        pv = ps_m.tile([128, 130], F32, tag="m")
                        nc.tensor.matmul(
                            pv[:], hTf[:, r, cc * 128:(cc + 1) * 128], w_v)
                        with nc.allow_low_precision(reason="v fp16"):
                            nc.vector.tensor_tensor(v_sb[:, c, :], pv[:], c_bv,
                                                    ALU.add)

                # attention: per-chunk groups with 1-group QK lookahead so
                # QK(c+1) overlaps exp(c); exp split across Act / DVE
                oT = sb.tile([128, NL], F16, tag="oT")
                po = [ps_o.tile([65, NL], F32, tag="o_acc", name=f"po{li}_{h}")
                      for h in range(H)]

                def emit_qk(c):
                    r, cc = c // 4, c % 4
                    ts = []
                    for h in range(H):
                        hs = slice(h * 64, (h + 1) * 64)
                        t = ps_s.tile([128, NL], F32, tag="scr",
                                      name=f"scr{li}_{c}_{h}")
                        nc.tensor.matmul(t[:],
                                         kT_sb[hs, r, cc * 128:(cc + 1) * 128],
                                         qT[hs, :])
                        ts.append(t)
                    return ts

                pscr_next = emit_qk(0)
                for c in range(NCH):
                    pscr = pscr_next
                    exs = []
                    for h in range(H):
                        if h == 0 or (c % 4 == 3 and c < 24):
                            ex = expp.tile([128, NL], F16, tag="exp",
                                           name=f"ex{li}_{c}_{h}")
                            nc.scalar.activation(ex[:], pscr[h][:], AF.Exp,
                                                 scale=0.125)
                            exs.append(ex[:])
                        else:
                            exi = expp.tile([128, NL], I16, tag="exp",
                                            name=f"ex{li}_{c}_{h}")
                            nc.vector.tensor_scalar(exi[:], pscr[h][:], FE_A,
                                                    FE_B, ALU.mult, ALU.add)
                            exs.append(exi[:].bitcast(F16))
                    if c + 1 < NCH:
                        pscr_next = emit_qk(c + 1)
                    for h in range(H):
                        nc.tensor.matmul(
                            po[h][:], v_sb[:, c, 65 * h:65 * h + 65],
                            exs[h][:], start=(c == 0), stop=(c == NCH - 1))
                # attention normalize: per-head recip + broadcast (unchunked)
                pdens = []
                for h in range(H):
                    den32 = sb.tile([1, NL], F32, tag="den32",
                                    name=f"den{li}_{h}")
                    nc.scalar.copy(den32[:], po[h][64:65, :])
                    rden32 = sb.tile([1, NL], F32, tag="rden32",
                                     name=f"rden{li}_{h}")
                    nc.vector.reciprocal_approx_fast(rden32[:], den32[:])
                    den16 = sb.tile([1, NL], F16, tag="den16",
                                    name=f"rden16_{li}_{h}")
                    with nc.allow_low_precision(reason="attn denom fp16"):
                        nc.scalar.copy(den16[:], rden32[:])
                    pden = ps_s.tile([64, NL], F32, tag="scr",
                                     name=f"pden{li}_{h}")
                    nc.tensor.matmul(pden[:], c_ones1[:], den16[:])
                    denB = sb.tile([64, NL], F16, tag="denB",
                                   name=f"denB{li}_{h}")
                    with nc.allow_low_precision(reason="denB fp16"):
                        nc.vector.tensor_copy(denB[:], pden[:])
                    pdens.append(denB)

                # ---- pipelined tail: 2 column chunks, stage-interleaved ----
                h1_16 = sb.tile([128, NL], F16, tag="h1_16")
                h1_32 = sb.tile([128, NL], F32, tag="h1_32")
                h2_16 = sb.tile([128, NL], F16, tag="hT16")
                h2_32 = sb.tile([128, NL], F32, tag="h2_32")
                h2_8 = None
                if li == 0:
                    h2_8 = sb.tile([128, NL], F8, tag="hT8")

                COLS = [slice(0, CK), slice(CK, 2 * CK)]
                t1c, ln1, zt, t2c, ln2 = {}, {}, {}, {}, {}

                for ck in range(2):
                    cols = COLS[ck]
                    with nc.allow_low_precision(reason="attn normalize fp16"):
                        for h in range(H):
                            hs = slice(h * 64, (h + 1) * 64)
                            nc.vector.tensor_tensor(oT[hs, cols],
                                                    po[h][0:64, cols],
                                                    pdens[h][:, cols],
                                                    ALU.mult)
                for ck in range(2):
                    cols = COLS[ck]
                    pool, ptag = pool_for(ck)
                    ps_p = pool.tile([128, CK], F32, tag=ptag,
                                     name=f"pp{li}{ck}")
                    nc.tensor.matmul(ps_p[:], w_o, oT[:, cols])
                    t1 = sb.tile([128, CK], F32, tag=f"resid1_{ck}")
                    nc.vector.scalar_tensor_tensor(t1[:], ps_p[:], c_bo,
                                                   hT32_in[:, cols],
                                                   ALU.add, ALU.add)
                    t1c[ck] = t1
                for ck in range(2):
                    ln1[ck] = ln_stage1(t1c[ck], ck, f"a{li}")
                for ck in range(2):
                    ln_stage2(ln1[ck], ck, f"a{li}")
                for ck in range(2):
                    ln_stage3(ln1[ck], ck, f"a{li}", c_ln[0], c_lnn[0])
                for ck in range(2):
                    ln_stage4(ln1[ck], ck, h1_32, h1_16, None, COLS[ck])
                # FF: sigmoid = 0.5*tanh(0.5x+0.5*bf1)+0.5 folded into 0.5*W2
                for ck in range(2):
                    cols = COLS[ck]
                    pool, ptag = pool_for(ck)
                    z = sb.tile([128, 2, CK], F16, tag=f"z_{ck}")
                    for f in range(2):
                        pz = pool.tile([128, CK], F32, tag=ptag,
                                       name=f"pz{li}{ck}{f}")
                        nc.tensor.matmul(pz[:], w_1[:, f * 128:(f + 1) * 128],
                                         h1_16[:, cols])
                        nc.scalar.activation(z[:, f, :], pz[:], AF.Tanh,
                                             bias=c_bf1[:, f:f + 1], scale=0.5)
                    zt[ck] = z
                for ck in range(2):
                    cols = COLS[ck]
                    pool, ptag = pool_for(ck)
                    ps_f = pool.tile([128, CK], F32, tag=ptag,
                                     name=f"pf{li}{ck}")
                    nc.tensor.matmul(ps_f[:], w_2[0], zt[ck][:, 0, :],
                                     start=True, stop=False)
                    nc.tensor.matmul(ps_f[:], w_2[1], zt[ck][:, 1, :],
                                     start=False, stop=False)
                    nc.tensor.matmul(ps_f[:], c_w2s, c_ln[0][0:1, 256:256 + CK],
                                     start=False, stop=True)
                    t2 = sb.tile([128, CK], F32, tag=f"resid2_{ck}")
                    nc.vector.scalar_tensor_tensor(t2[:], ps_f[:], c_bf2,
                                                   h1_32[:, cols],
                                                   ALU.add, ALU.add)
                    t2c[ck] = t2
                for ck in range(2):
                    ln2[ck] = ln_stage1(t2c[ck], ck, f"b{li}")
                for ck in range(2):
                    ln_stage2(ln2[ck], ck, f"b{li}")
                for ck in range(2):
                    ln_stage3(ln2[ck], ck, f"b{li}", c_ln[1], c_lnn[1])
                for ck in range(2):
                    ln_stage4(ln2[ck], ck, h2_32, h2_16, h2_8, COLS[ck])
                    if tail_hook is not None:
                        tail_hook(ck, h2_16)
                return h2_32, h2_16, h2_8

            # conv: outT = relu(A' @ (h @ Wh) + bh), partial A-matmuls woven
            # into layer-2's tail as h2_16 column chunks complete
            conv = {}

            def conv_hook(ck, h16t):
                if ck == 0:
                    xt_sb = sb.tile([128, 4, 128], F16, tag="xt_loc")
                    conv["xt"] = xt_sb
                    for c in (0, 1):
                        px = ps_m.tile([128, 128], F32, tag="m", name=f"px{c}")
                        nc.tensor.matmul(px[:], h16t[:, c * 128:(c + 1) * 128],
                                         w_h)
                        nc.vector.tensor_copy(xt_sb[:, c, :], px[:])
                    ppA = [ps_s.tile([128, NL], F32, tag="scr", name=f"ppA{b}")
                           for b in range(4)]
                    conv["ppA"] = ppA
                    for b in range(4):
                        for c in (0, 1):
                            nc.tensor.matmul(ppA[b][:], xt_sb[:, c, :],
                                             at16[:, c, b * NL:(b + 1) * NL],
                                             start=(c == 0), stop=False)
                    return
                xt_sb = conv["xt"]
                for c in (2, 3):
                    px = ps_m.tile([128, 128], F32, tag="m", name=f"px{c}")
                    nc.tensor.matmul(px[:], h16t[:, c * 128:(c + 1) * 128], w_h)
                    nc.vector.tensor_copy(xt_sb[:, c, :], px[:])
                partial = sb.tile([128, W, NL], F16, tag="partial")
                ppA = conv["ppA"]
                for b in range(4):
                    for c in (2, 3):
                        nc.tensor.matmul(ppA[b][:], xt_sb[:, c, :],
                                         at16[:, c, b * NL:(b + 1) * NL],
                                         start=False, stop=(c == 3))
                ppB = [ps_s.tile([128, NL], F32, tag="scr", name=f"ppB{b}")
                       for b in range(4)]
                for b in range(4):
                    with nc.allow_low_precision(reason="conv partial fp16"):
                        if b % 2 == 0:
                            nc.scalar.copy(partial[:, b, :], ppA[b][:])
                        else:
                            nc.vector.tensor_copy(partial[:, b, :], ppA[b][:])
                    nc.sync.dma_start(
                        rs_in[b, :].rearrange("(k p j) -> p k j", p=128, k=2),
                        partial[:, b, :])
                for b in range(4):
                    for c in range(4):
                        nc.tensor.matmul(ppB[b][:], xt_sb[:, c, :],
                                         at16[:, c, (4 + b) * NL:(5 + b) * NL],
                                         start=(c == 0), stop=(c == 3))
                for b in range(4):
                    with nc.allow_low_precision(reason="conv partial fp16"):
                        if b % 2 == 0:
                            nc.scalar.copy(partial[:, 4 + b, :], ppB[b][:])
                        else:
                            nc.vector.tensor_copy(partial[:, 4 + b, :],
                                                  ppB[b][:])
                    nc.sync.dma_start(
                        rs_in[4 + b, :].rearrange("(k p j) -> p k j",
                                                  p=128, k=2),
                        partial[:, 4 + b, :])

            h32, h16, h8 = mha_ff_layer(0, hT32, hT16, hTf0, None)
            h32, h16, _ = mha_ff_layer(1, h32, h16, None, h8,
                                       tail_hook=conv_hook)

            nc.gpsimd.collective_compute(
                "ReduceScatter", ALU.add, replica_groups=RG,
                ins=[rs_in[:, :]], outs=[rs_out[:]])

            convo = sb.tile([128, NL], F16, tag="convo")
            res = sb.tile([128, NL], F32, tag="res")
            K2 = NL // 2
            for k in range(2):
                nc.sync.dma_start(
                    convo[:, k * K2:(k + 1) * K2],
                    rs_out[k, :].rearrange("(p j) -> p j", p=128))
            for k in range(2):
                nc.scalar.activation(res[:, k * K2:(k + 1) * K2],
                                     convo[:, k * K2:(k + 1) * K2],
                                     AF.Relu, bias=c_bh)
                nc.sync.dma_start(out_t[k, :, :], res[:, k * K2:(k + 1) * K2])

    nc.compile()
    return nc


_NC = None


def _get_nc():
    global _NC
    if _NC is None:
        _NC = build_program()
    return _NC


def make_in_maps(inputs):
    import ml_dtypes
    F8NP = ml_dtypes.float8_e4m3

    x = np.asarray(inputs["x"], dtype=np.float32)
    edge = np.asarray(inputs["edge"])
    gw = {k: np.asarray(inputs[k], dtype=np.float32) for k in
          ("Wq", "bq", "Wk", "bk", "Wv", "bv", "Wo", "bo", "g_ln1", "b_ln1",
           "W1", "bf1", "W2", "bf2", "g_ln2", "b_ln2", "Wh", "bh")}

    node_idx = np.asarray(edge[0], dtype=np.int64)
    he_idx = np.asarray(edge[1], dtype=np.int64)
    counts = np.zeros((N, NE), dtype=np.float32)
    np.add.at(counts, (node_idx, he_idx), 1.0)
    Bdeg = counts.sum(axis=0)
    Ddeg = counts.sum(axis=1)
    Binv = np.where(Bdeg > 0, 1.0 / np.maximum(Bdeg, 1), 0.0).astype(np.float32)
    Dinv = np.where(Ddeg > 0, 1.0 / np.maximum(Ddeg, 1), 0.0).astype(np.float32)

    # S = H Binv H^T (symmetric); per-core slice of A'^T = S[rows,:] * Dinv
    S = (counts * Binv[None, :]) @ counts.T

    wqkv = np.zeros((128, 386), dtype=np.float16)
    wqkv[:, 0:128] = gw["Wq"]
    wqkv[:, 128:256] = gw["Wk"]
    wqkv[:, 256:320] = gw["Wv"][:, 0:64]
    wqkv[:, 321:385] = gw["Wv"][:, 64:128]
    wf16 = np.zeros((128, _WF16_COLS), dtype=np.float16)
    wf16[:, _WO:_WO + 128] = gw["Wo"]
    wf16[:, _W1:_W1 + FF] = gw["W1"]
    wf16[:, _W2:_W2 + 256] = (
        (0.5 * gw["W2"]).reshape(2, 128, 128).transpose(1, 0, 2).reshape(128, 256))
    wf16[:, _WH:_WH + 128] = gw["Wh"]
    wf16[:, _ID:_ID + 128] = np.eye(128, dtype=np.float16)
    wf16[:, _ONE:_ONE + 1] = 1.0 / 128

    wb32 = np.zeros((128, _WB32_COLS), dtype=np.float32)
    wb32[:, _BQ] = gw["bq"]
    wb32[:, _BKV] = gw["bk"]
    wb32[:, _BKV + 1:_BKV + 65] = gw["bv"][0:64][None, :]
    wb32[:, _BKV + 66:_BKV + 130] = gw["bv"][64:128][None, :]
    wb32[:, _BKV + 65] = 1.0
    wb32[:, _BKV + 130] = 1.0
    wb32[:, _BO] = gw["bo"]
    wb32[:, _BF2] = gw["bf2"]
    wb32[:, _BH] = gw["bh"]
    wb32[:, _BF1:_BF1 + 2] = (0.5 * gw["bf1"]).reshape(2, 128).T

    def lnrows(g, b):
        r = np.zeros(768, dtype=np.float16)
        r[0:128] = g
        r[128:256] = b
        r[256:768] = 1.0
        return r

    lnr = np.zeros((1, _LNR_COLS), dtype=np.float16)
    lnr[0, _LN1R:_LN1R + 768] = lnrows(gw["g_ln1"], gw["b_ln1"])
    lnr[0, _LN2R:_LN2R + 768] = lnrows(gw["g_ln2"], gw["b_ln2"])
    lnr[0, _W2S:_W2S + 128] = 0.5 * gw["W2"].sum(axis=0)
    lnr[0, _LN1N:_LN1N + 128] = -gw["g_ln1"]
    lnr[0, _LN2N:_LN2N + 128] = -gw["g_ln2"]

    xTf8 = np.ascontiguousarray(
        x.T.astype(np.float16).reshape(128, W, NL)).astype(F8NP)

    shared = dict(WQKV=wqkv, WF16=np.ascontiguousarray(wf16[:, 386:]),
                  WB32=wb32, LNR=lnr, xTf8=xTf8)

    in_maps = []
    for r in range(W):
        rows = slice(r * NL, (r + 1) * NL)
        xT = np.ascontiguousarray(x[rows, :].T)
        at = np.ascontiguousarray(
            (S[rows, :] * Dinv[None, :]).astype(np.float16)
            .reshape(4, 128, N).transpose(1, 0, 2))
        m = dict(shared)
        m.update(xT32=xT.astype(np.float32), xT16=xT.astype(np.float16),
                 AT=at)
        in_maps.append(m)
    return in_maps


def kernel(**inputs) -> np.ndarray:
    nc = _get_nc()
    in_maps = make_in_maps(inputs)
    res = run_bass_kernel_spmd(nc, in_maps, core_ids=list(range(W)))
    out = np.empty((N, D), dtype=np.float32)
    for r in range(W):
        o = res.results[r]["outT"]          # [2, 128, NL//2]
        out[r * NL:(r + 1) * NL, :] = (
            o.transpose(1, 0, 2).reshape(128, NL).T)
    return out


if __name__ == "__main__":
    build_program()
    print("build OK")
